# revision 17
# baseline (speedup 1.0000x reference)
"""Causal self-attention on 8 TRN2 NeuronCores.

Problem: B=4, S=2048, D=1024, H=16 heads (hd=64), fp32 in/out.
  qkv = x @ w_qkv + b_qkv ; causal softmax attention ; y @ w_out + b_out

Sharding (tensor-parallel over heads x data-parallel over batch):
  core c -> batch b = c//2, head-group hg = c%2 (8 heads each).
  Each core computes qkv for its 8 heads from x[b], runs attention, and
  produces a partial output  y_local @ w_out[rows]  of shape [S, D].
  Host unshards: out[b] = partial[2b] + partial[2b+1] + b_out.

Device kernel (per core), bf16 matmul operands / fp32 PSUM accumulation:
  - x passed transposed (xT [D, S], bf16) so both projections contract D
    on partitions with no device-side transposes.
  - q,k produced directly transposed (qT/kT [64, S] per head) via
    out = w.T @ x; heads processed in pairs packed at partition offsets
    0-63 / 64-127.  v in natural layout with a ones column (v_aug) so
    the PV matmul also produces the softmax denominator.
  - scores computed transposed (S_T[k, q]) so attT = exp(S_T) is already
    in PV layout; no attention-matrix transposes.  Causal handled by
    block skipping + additive -1e30 mask windows on diagonal tiles;
    fully-masked halves of the 1024-wide attT tiles are zero-memset.
  - softmax denominator: ones column accumulates rowsum into row 64 of
    the PV psum; reciprocal_approx_fast + a one-hot K=64 f32r broadcast
    matmul spreads 1/rowsum across partitions; normalization fuses into
    the psum->SBUF eviction of yT.  Max-subtraction is skipped (scores
    are O(1) by construction; exp exact in fp32 range).
"""

import os
import sys

for _p in ("/root/.axon_site/_ro/trn_rl_repo", "/opt/trn_rl_repo"):
    if os.path.isdir(_p) and _p not in sys.path:
        sys.path.append(_p)

import ml_dtypes
import numpy as np

import concourse.bass as bass  # noqa: F401
import concourse.mybir as mybir
import concourse.tile as tile
from concourse import bacc
from concourse.bass_utils import run_bass_kernel_spmd

B, S, D, H = 4, 2048, 1024, 16
HD = 64
HPC = 8          # heads per core
NPAIR = HPC // 2
KO = D // 128    # contraction chunks over D
ATT_SCALE = 1.0 / np.sqrt(HD)
NEG = -1.0e30

F32 = mybir.dt.float32
F32R = mybir.dt.float32r
BF16 = mybir.dt.bfloat16
NPBF16 = ml_dtypes.bfloat16


def build_nc(S_=S):
    KT = S_ // 128    # k tiles
    TB = S_ // 512    # token blocks for projections

    nc = bacc.Bacc(None)
    xT_d = nc.dram_tensor("xT", [D, S_], BF16, kind="ExternalInput")
    wqk_d = nc.dram_tensor("wqk", [D, NPAIR, 2, 128], BF16, kind="ExternalInput")
    bqk_d = nc.dram_tensor("bqk", [128, NPAIR, 2], F32, kind="ExternalInput")
    wv_d = nc.dram_tensor("wv", [D, HPC * HD], BF16, kind="ExternalInput")
    bv_d = nc.dram_tensor("bv", [128, HPC * HD], F32, kind="ExternalInput")
    wout_d = nc.dram_tensor("wout", [HPC * HD, D], BF16, kind="ExternalInput")
    mask_d = nc.dram_tensor("mask", [128, 896], F32, kind="ExternalInput")
    out_d = nc.dram_tensor("out", [S_, D], F32, kind="ExternalOutput")

    with tile.TileContext(nc) as tc, nc.allow_low_precision("bf16/f32r matmul operands"):
        with (
            tc.tile_pool(name="const", bufs=1) as constp,
            tc.tile_pool(name="psA", bufs=2, space="PSUM") as psA,
            tc.tile_pool(name="psS", bufs=2, space="PSUM") as psS,
            tc.tile_pool(name="psY", bufs=4, space="PSUM") as psY,
        ):
            mask_sb = constp.tile([128, 896], F32)
            nc.sync.dma_start(mask_sb[:], mask_d[:])
            bqk_sb = constp.tile([128, NPAIR, 2], F32)
            nc.sync.dma_start(bqk_sb[:], bqk_d[:])
            bv_sb = constp.tile([128, HPC * HD], F32)
            nc.sync.dma_start(bv_sb[:], bv_d[:])
            # v with ones column (col 64); col 65 is pad
            vaug = constp.tile([128, KT, HPC, 66], BF16)
            nc.gpsimd.memset(vaug[:, :, :, 64], 1.0)
            yT = constp.tile([128, NPAIR, S_], BF16)

            with tc.tile_pool(name="px", bufs=1) as px:
                xT = px.tile([128, KO, S_], BF16)
                xr = xT_d.rearrange("(ko p) t -> p ko t", p=128)
                for i in range(4):
                    nc.sync.dma_start(xT[:, 2 * i : 2 * i + 2, :], xr[:, 2 * i : 2 * i + 2, :])

                # ---- v projection (all heads), biased, into v_aug ----
                with tc.tile_pool(name="pwv", bufs=1) as pwv:
                    wv_sb = pwv.tile([128, KO, HPC * HD], BF16)
                    nc.sync.dma_start(wv_sb[:], wv_d.rearrange("(ko p) c -> p ko c", p=128))
                    for tt in range(KT):
                        ps = psA.tile([128, 512], F32, tag="psA")
                        for k in range(KO):
                            nc.tensor.matmul(
                                ps,
                                xT[:, k, tt * 128 : (tt + 1) * 128],
                                wv_sb[:, k, :],
                                start=(k == 0),
                                stop=(k == KO - 1),
                            )
                        nc.vector.tensor_tensor(
                            vaug[:, tt, :, 0:64],
                            ps[:].rearrange("p (h d) -> p h d", h=HPC),
                            bv_sb[:].rearrange("p (h d) -> p h d", h=HPC),
                            mybir.AluOpType.add,
                        )

                with (
                    tc.tile_pool(name="pqk", bufs=2) as pqk,
                    tc.tile_pool(name="pw", bufs=2) as pw,
                    tc.tile_pool(name="patt", bufs=3) as patt,
                    tc.tile_pool(name="pnorm", bufs=2) as pnorm,
                ):
                    for pr in range(NPAIR):
                        # ---- q/k projection for head pair, packed 64|64 ----
                        wqk_sb = pw.tile([128, KO, 2, 128], BF16, tag="wqk")
                        nc.sync.dma_start(
                            wqk_sb[:],
                            wqk_d.rearrange("(ko p) r c2 c -> p ko r c2 c", p=128)[
                                :, :, pr, :, :
                            ],
                        )
                        qT = pqk.tile([128, S_], BF16, tag="qT")
                        kT = pqk.tile([128, S_], BF16, tag="kT")
                        for cqk in range(2):
                            dst = qT if cqk == 0 else kT
                            for tb0 in range(0, TB, 2):
                                tbs = [tb0] + ([tb0 + 1] if tb0 + 1 < TB else [])
                                pst = [
                                    psA.tile([128, 512], F32, tag="psA", name=f"pj{i}")
                                    for i in range(len(tbs))
                                ]
                                for k in range(KO):
                                    for i, tb in enumerate(tbs):
                                        nc.tensor.matmul(
                                            pst[i],
                                            wqk_sb[:, k, cqk, :],
                                            xT[:, k, tb * 512 : (tb + 1) * 512],
                                            start=(k == 0),
                                            stop=(k == KO - 1),
                                        )
                                for i, tb in enumerate(tbs):
                                    nc.vector.tensor_scalar_add(
                                        dst[:, tb * 512 : (tb + 1) * 512],
                                        pst[i][:],
                                        bqk_sb[:, pr, cqk : cqk + 1],
                                    )

                        # ---- attention for both heads of the pair ----
                        for a in range(S_ // 512):
                            psy = [None, None]
                            for h01 in range(2):
                                psy[h01] = psY.tile(
                                    [65, 512], F32, tag="psY", name=f"psy{h01}"
                                )
                            nj = 4 * a + 4
                            for j in range(nj):
                                o = 128 * j - 512 * a
                                # phase-grouped so the two K=64 score matmuls
                                # sit adjacent in the PE queue and pack onto
                                # disjoint row halves of the array
                                pss2, att2 = [], []
                                for h01 in range(2):
                                    lo, hi = h01 * 64, h01 * 64 + 64
                                    pss = psS.tile(
                                        [128, 512], F32, tag="psS", name=f"pss{h01}"
                                    )
                                    nc.tensor.matmul(
                                        pss,
                                        kT[lo:hi, j * 128 : (j + 1) * 128],
                                        qT[lo:hi, a * 512 : (a + 1) * 512],
                                        start=True,
                                        stop=True,
                                    )
                                    pss2.append(pss)
                                for h01 in range(2):
                                    if o >= 0:
                                        nc.vector.tensor_tensor(
                                            pss2[h01][:],
                                            pss2[h01][:],
                                            mask_sb[:, 384 - o : 896 - o],
                                            mybir.AluOpType.add,
                                        )
                                    att = patt.tile(
                                        [128, 512], BF16, tag="att", name=f"att{h01}"
                                    )
                                    nc.scalar.activation(
                                        att[:],
                                        pss2[h01][:],
                                        mybir.ActivationFunctionType.Exp,
                                        scale=float(ATT_SCALE),
                                    )
                                    att2.append(att)
                                for h01 in range(2):
                                    nc.tensor.matmul(
                                        psy[h01],
                                        vaug[:, j, 2 * pr + h01, 0:65],
                                        att2[h01][:],
                                        start=(j == 0),
                                        stop=(j == nj - 1),
                                    )
                            # ---- normalize + write yT ----
                            for h01 in range(2):
                                # 1/rowsum = exp(-ln(rowsum)) on ACT (keeps the
                                # slow 1-lane reciprocal off the in-order DVE)
                                rtmp = pnorm.tile([65, 512], F32, tag="rt")
                                nc.scalar.activation(
                                    rtmp[64:65, :],
                                    psy[h01][64:65, :],
                                    mybir.ActivationFunctionType.Ln,
                                )
                                nc.scalar.activation(
                                    rtmp[64:65, :],
                                    rtmp[64:65, :],
                                    mybir.ActivationFunctionType.Exp,
                                    scale=-1.0,
                                )
                                rr0 = pnorm.tile([1, 512], F32, tag="rr0")
                                nc.sync.dma_start(rr0[:], rtmp[64:65, :])
                                bc = pnorm.tile([64, 512], F32, tag="bc")
                                nc.gpsimd.partition_broadcast(bc[:], rr0[:])
                                dsts = a * 512
                                if h01 == 0:
                                    nc.vector.tensor_tensor(
                                        yT[0:64, pr, dsts : dsts + 512],
                                        psy[h01][0:64, :],
                                        bc[:],
                                        mybir.AluOpType.mult,
                                    )
                                else:
                                    stg = pnorm.tile([64, 512], BF16, tag="stg")
                                    nc.vector.tensor_tensor(
                                        stg[:],
                                        psy[h01][0:64, :],
                                        bc[:],
                                        mybir.AluOpType.mult,
                                    )
                                    nc.sync.dma_start(
                                        yT[64:128, pr, dsts : dsts + 512], stg[:]
                                    )

            # ---- output projection: partial = yT.T @ w_out ----
            with tc.tile_pool(name="pout", bufs=1) as pout, tc.tile_pool(
                name="postage", bufs=3
            ) as postage:
                wout_sb = pout.tile([128, NPAIR, D], BF16)
                nc.sync.dma_start(wout_sb[:], wout_d.rearrange("(cc p) c -> p cc c", p=128))
                for tt in range(S_ // 128):
                    for nh in range(2):
                        ps = psA.tile([128, 512], F32, tag="psA")
                        for cc in range(NPAIR):
                            nc.tensor.matmul(
                                ps,
                                yT[:, cc, tt * 128 : (tt + 1) * 128],
                                wout_sb[:, cc, nh * 512 : (nh + 1) * 512],
                                start=(cc == 0),
                                stop=(cc == NPAIR - 1),
                            )
                        ot = postage.tile([128, 512], F32, tag="ot")
                        nc.vector.tensor_copy(ot[:], ps[:])
                        nc.sync.dma_start(
                            out_d[tt * 128 : (tt + 1) * 128, nh * 512 : (nh + 1) * 512], ot[:]
                        )

    nc.finalize()
    return nc


def make_host_inputs(x, w_qkv, b_qkv, w_out, b_out, S_=S):
    """Build the 8 per-core input maps (host-side shard/pack/cast)."""
    x = np.asarray(x, dtype=np.float32)
    w_qkv = np.asarray(w_qkv, dtype=np.float32)
    b_qkv = np.asarray(b_qkv, dtype=np.float32)
    w_out = np.asarray(w_out, dtype=np.float32)

    mask = np.where(
        np.arange(896)[None, :] >= np.arange(128)[:, None] + 384, 0.0, NEG
    ).astype(np.float32)

    per_hg = {}
    for hg in range(2):
        wqk = np.empty((D, NPAIR, 2, 128), np.float32)
        bqk = np.empty((128, NPAIR, 2), np.float32)
        for p in range(NPAIR):
            h0, h1 = hg * HPC + 2 * p, hg * HPC + 2 * p + 1
            wqk[:, p, 0, 0:64] = w_qkv[:, h0 * HD : (h0 + 1) * HD]
            wqk[:, p, 0, 64:128] = w_qkv[:, h1 * HD : (h1 + 1) * HD]
            wqk[:, p, 1, 0:64] = w_qkv[:, D + h0 * HD : D + (h0 + 1) * HD]
            wqk[:, p, 1, 64:128] = w_qkv[:, D + h1 * HD : D + (h1 + 1) * HD]
            bqk[0:64, p, 0] = b_qkv[h0 * HD : (h0 + 1) * HD]
            bqk[64:128, p, 0] = b_qkv[h1 * HD : (h1 + 1) * HD]
            bqk[0:64, p, 1] = b_qkv[D + h0 * HD : D + (h0 + 1) * HD]
            bqk[64:128, p, 1] = b_qkv[D + h1 * HD : D + (h1 + 1) * HD]
        wv = w_qkv[:, 2 * D + hg * 512 : 2 * D + (hg + 1) * 512]
        bv = np.broadcast_to(
            b_qkv[2 * D + hg * 512 : 2 * D + (hg + 1) * 512], (128, 512)
        ).copy()
        wout = w_out[hg * 512 : (hg + 1) * 512, :]
        per_hg[hg] = dict(
            wqk=np.ascontiguousarray(wqk.astype(NPBF16)),
            bqk=bqk,
            wv=np.ascontiguousarray(wv.astype(NPBF16)),
            bv=bv,
            wout=np.ascontiguousarray(wout.astype(NPBF16)),
        )

    xT_by_b = [
        np.ascontiguousarray(x[b, :S_].T.astype(NPBF16)) for b in range(B)
    ]
    in_maps = []
    for c in range(8):
        b, hg = c // 2, c % 2
        m = dict(per_hg[hg])
        m["xT"] = xT_by_b[b]
        m["mask"] = mask
        in_maps.append(m)
    return in_maps


_NC_CACHE = {}


def _get_nc(S_=S):
    if S_ not in _NC_CACHE:
        _NC_CACHE[S_] = build_nc(S_)
    return _NC_CACHE[S_]


def kernel(x, w_qkv, b_qkv, w_out, b_out):
    x = np.asarray(x, dtype=np.float32)
    b_out = np.asarray(b_out, dtype=np.float32)
    in_maps = make_host_inputs(x, w_qkv, b_qkv, w_out, b_out)
    nc = _get_nc()
    res = run_bass_kernel_spmd(nc, in_maps, list(range(8))).results
    out = np.empty((B, S, D), np.float32)
    for b in range(B):
        out[b] = res[2 * b]["out"] + res[2 * b + 1]["out"] + b_out[None, :]
    return out


# revision 19
# speedup vs baseline: 1.2803x; 1.2803x over previous
"""Causal self-attention on 8 TRN2 NeuronCores.

Problem: B=4, S=2048, D=1024, H=16 heads (hd=64), fp32 in/out.
  qkv = x @ w_qkv + b_qkv ; causal softmax attention ; y @ w_out + b_out

Sharding (tensor-parallel over heads x data-parallel over batch):
  core c -> batch b = c//2, head-group hg = c%2 (8 heads each).
  Each core computes qkv for its 8 heads from x[b], runs attention, and
  produces a partial output  y_local @ w_out[rows]  of shape [S, D].
  Host unshards: out[b] = partial[2b] + partial[2b+1] + b_out.

Device kernel (per core), bf16 matmul operands / fp32 PSUM accumulation:
  - x passed transposed (xT [D, S], bf16) so both projections contract D
    on partitions with no device-side transposes.
  - q,k produced directly transposed (qT/kT [64, S] per head) via
    out = w.T @ x; heads processed in pairs packed at partition offsets
    0-63 / 64-127.  v in natural layout with a ones column (v_aug) so
    the PV matmul also produces the softmax denominator.
  - scores computed transposed (S_T[k, q]) so attT = exp(S_T) is already
    in PV layout; no attention-matrix transposes.  Causal handled by
    block skipping + additive -1e30 mask windows on diagonal tiles;
    fully-masked halves of the 1024-wide attT tiles are zero-memset.
  - softmax denominator: ones column accumulates rowsum into row 64 of
    the PV psum; reciprocal_approx_fast + a one-hot K=64 f32r broadcast
    matmul spreads 1/rowsum across partitions; normalization fuses into
    the psum->SBUF eviction of yT.  Max-subtraction is skipped (scores
    are O(1) by construction; exp exact in fp32 range).
"""

import os
import sys

for _p in ("/root/.axon_site/_ro/trn_rl_repo", "/opt/trn_rl_repo"):
    if os.path.isdir(_p) and _p not in sys.path:
        sys.path.append(_p)

import ml_dtypes
import numpy as np

import concourse.bass as bass  # noqa: F401
import concourse.mybir as mybir
import concourse.tile as tile
from concourse import bacc
from concourse.bass_utils import run_bass_kernel_spmd

B, S, D, H = 4, 2048, 1024, 16
HD = 64
HPC = 8          # heads per core
NPAIR = HPC // 2
KO = D // 128    # contraction chunks over D
ATT_SCALE = 1.0 / np.sqrt(HD)
NEG = -1.0e30

F32 = mybir.dt.float32
F32R = mybir.dt.float32r
BF16 = mybir.dt.bfloat16
NPBF16 = ml_dtypes.bfloat16


def build_nc(S_=S):
    KT = S_ // 128    # k tiles
    TB = S_ // 512    # token blocks for projections

    nc = bacc.Bacc(None)
    xT_d = nc.dram_tensor("xT", [D, S_], BF16, kind="ExternalInput")
    wqk_d = nc.dram_tensor("wqk", [D, NPAIR, 2, 128], BF16, kind="ExternalInput")
    bqk_d = nc.dram_tensor("bqk", [128, NPAIR, 2], F32, kind="ExternalInput")
    wv_d = nc.dram_tensor("wv", [D, HPC * HD], BF16, kind="ExternalInput")
    bv_d = nc.dram_tensor("bv", [128, HPC * HD], F32, kind="ExternalInput")
    wout_d = nc.dram_tensor("wout", [HPC * HD, D], BF16, kind="ExternalInput")
    mask_d = nc.dram_tensor("mask", [128, 896], F32, kind="ExternalInput")
    out_d = nc.dram_tensor("out", [S_, D], F32, kind="ExternalOutput")

    with tile.TileContext(nc) as tc, nc.allow_low_precision("bf16/f32r matmul operands"):
        with (
            tc.tile_pool(name="const", bufs=1) as constp,
            tc.tile_pool(name="psA", bufs=2, space="PSUM") as psA,
            tc.tile_pool(name="psS", bufs=4, space="PSUM") as psS,
            tc.tile_pool(name="psY", bufs=2, space="PSUM") as psY,
        ):
            mask_sb = constp.tile([128, 896], F32)
            nc.sync.dma_start(mask_sb[:], mask_d[:])
            bqk_sb = constp.tile([128, NPAIR, 2], F32)
            nc.sync.dma_start(bqk_sb[:], bqk_d[:])
            bv_sb = constp.tile([128, HPC * HD], F32)
            nc.sync.dma_start(bv_sb[:], bv_d[:])
            # v with ones column (col 64); col 65 is pad
            vaug = constp.tile([128, KT, HPC, 66], BF16)
            nc.gpsimd.memset(vaug[:, :, :, 64], 1.0)
            yT = constp.tile([128, NPAIR, S_], BF16)

            with tc.tile_pool(name="px", bufs=1) as px:
                xT = px.tile([128, KO, S_], BF16)
                xr = xT_d.rearrange("(ko p) t -> p ko t", p=128)
                for i in range(4):
                    nc.sync.dma_start(xT[:, 2 * i : 2 * i + 2, :], xr[:, 2 * i : 2 * i + 2, :])

                # ---- v projection (all heads), biased, into v_aug ----
                with tc.tile_pool(name="pwv", bufs=1) as pwv:
                    wv_sb = pwv.tile([128, KO, HPC * HD], BF16)
                    nc.sync.dma_start(wv_sb[:], wv_d.rearrange("(ko p) c -> p ko c", p=128))
                    for tt in range(KT):
                        ps = psA.tile([128, 512], F32, tag="psA")
                        for k in range(KO):
                            nc.tensor.matmul(
                                ps,
                                xT[:, k, tt * 128 : (tt + 1) * 128],
                                wv_sb[:, k, :],
                                start=(k == 0),
                                stop=(k == KO - 1),
                            )
                        nc.vector.tensor_tensor(
                            vaug[:, tt, :, 0:64],
                            ps[:].rearrange("p (h d) -> p h d", h=HPC),
                            bv_sb[:].rearrange("p (h d) -> p h d", h=HPC),
                            mybir.AluOpType.add,
                        )

                with (
                    tc.tile_pool(name="pqk", bufs=2) as pqk,
                    tc.tile_pool(name="pw", bufs=2) as pw,
                    tc.tile_pool(name="patt", bufs=4) as patt,
                    tc.tile_pool(name="pnorm", bufs=2) as pnorm,
                ):
                    for pr in range(NPAIR):
                        # ---- q/k projection for head pair, packed 64|64 ----
                        wqk_sb = pw.tile([128, KO, 2, 128], BF16, tag="wqk")
                        nc.sync.dma_start(
                            wqk_sb[:],
                            wqk_d.rearrange("(ko p) r c2 c -> p ko r c2 c", p=128)[
                                :, :, pr, :, :
                            ],
                        )
                        qT = pqk.tile([128, S_], BF16, tag="qT")
                        kT = pqk.tile([128, S_], BF16, tag="kT")
                        for cqk in range(2):
                            dst = qT if cqk == 0 else kT
                            for tb0 in range(0, TB, 2):
                                tbs = [tb0] + ([tb0 + 1] if tb0 + 1 < TB else [])
                                pst = [
                                    psA.tile([128, 512], F32, tag="psA", name=f"pj{i}")
                                    for i in range(len(tbs))
                                ]
                                for k in range(KO):
                                    for i, tb in enumerate(tbs):
                                        nc.tensor.matmul(
                                            pst[i],
                                            wqk_sb[:, k, cqk, :],
                                            xT[:, k, tb * 512 : (tb + 1) * 512],
                                            start=(k == 0),
                                            stop=(k == KO - 1),
                                        )
                                for i, tb in enumerate(tbs):
                                    nc.vector.tensor_scalar_add(
                                        dst[:, tb * 512 : (tb + 1) * 512],
                                        pst[i][:],
                                        bqk_sb[:, pr, cqk : cqk + 1],
                                    )

                        # ---- attention for both heads of the pair ----
                        for a in range(S_ // 512):
                            psy = [None, None]
                            for h01 in range(2):
                                psy[h01] = psY.tile(
                                    [65, 512], F32, tag="psY", name=f"psy{h01}"
                                )
                            nj = 4 * a + 4
                            for j in range(nj):
                                o = 128 * j - 512 * a
                                # phase-grouped so the two K=64 score matmuls
                                # sit adjacent in the PE queue and pack onto
                                # disjoint row halves of the array
                                # diagonal tiles: only columns >= o are live;
                                # compute scores/mask/exp on the live strip and
                                # zero the rest of attT
                                oo = max(o, 0)
                                W = 512 - oo
                                pss2, att2 = [], []
                                for h01 in range(2):
                                    lo, hi = h01 * 64, h01 * 64 + 64
                                    pss = psS.tile(
                                        [128, 512], F32, tag="psS", name=f"pss{h01}"
                                    )
                                    nc.tensor.matmul(
                                        pss[:, 0:W],
                                        kT[lo:hi, j * 128 : (j + 1) * 128],
                                        qT[lo:hi, a * 512 + oo : (a + 1) * 512],
                                        start=True,
                                        stop=True,
                                    )
                                    pss2.append(pss)
                                for h01 in range(2):
                                    if o >= 0:
                                        nc.vector.tensor_tensor(
                                            pss2[h01][:, 0:W],
                                            pss2[h01][:, 0:W],
                                            mask_sb[:, 384 : 896 - oo],
                                            mybir.AluOpType.add,
                                        )
                                    att = patt.tile(
                                        [128, 512], BF16, tag="att", name=f"att{h01}"
                                    )
                                    if oo > 0:
                                        nc.gpsimd.memset(att[:, 0:oo], 0.0)
                                    nc.scalar.activation(
                                        att[:, oo:512],
                                        pss2[h01][:, 0:W],
                                        mybir.ActivationFunctionType.Exp,
                                        scale=float(ATT_SCALE),
                                    )
                                    att2.append(att)
                                for h01 in range(2):
                                    nc.tensor.matmul(
                                        psy[h01],
                                        vaug[:, j, 2 * pr + h01, 0:65],
                                        att2[h01][:],
                                        start=(j == 0),
                                        stop=(j == nj - 1),
                                    )
                            # ---- normalize + write yT ----
                            for h01 in range(2):
                                # 1/rowsum = exp(-ln(rowsum)) on ACT (keeps the
                                # slow 1-lane reciprocal off the in-order DVE)
                                rtmp = pnorm.tile([65, 512], F32, tag="rt")
                                nc.scalar.activation(
                                    rtmp[64:65, :],
                                    psy[h01][64:65, :],
                                    mybir.ActivationFunctionType.Ln,
                                )
                                nc.scalar.activation(
                                    rtmp[64:65, :],
                                    rtmp[64:65, :],
                                    mybir.ActivationFunctionType.Exp,
                                    scale=-1.0,
                                )
                                rr0 = pnorm.tile([1, 512], F32, tag="rr0")
                                nc.sync.dma_start(rr0[:], rtmp[64:65, :])
                                bc = pnorm.tile([64, 512], F32, tag="bc")
                                nc.gpsimd.partition_broadcast(bc[:], rr0[:])
                                dsts = a * 512
                                if h01 == 0:
                                    nc.vector.tensor_tensor(
                                        yT[0:64, pr, dsts : dsts + 512],
                                        psy[h01][0:64, :],
                                        bc[:],
                                        mybir.AluOpType.mult,
                                    )
                                else:
                                    stg = pnorm.tile([64, 512], BF16, tag="stg")
                                    nc.vector.tensor_tensor(
                                        stg[:],
                                        psy[h01][0:64, :],
                                        bc[:],
                                        mybir.AluOpType.mult,
                                    )
                                    nc.sync.dma_start(
                                        yT[64:128, pr, dsts : dsts + 512], stg[:]
                                    )

            # ---- output projection: partial = yT.T @ w_out ----
            with tc.tile_pool(name="pout", bufs=1) as pout, tc.tile_pool(
                name="postage", bufs=3
            ) as postage:
                wout_sb = pout.tile([128, NPAIR, D], BF16)
                nc.sync.dma_start(wout_sb[:], wout_d.rearrange("(cc p) c -> p cc c", p=128))
                for tt in range(S_ // 128):
                    for nh in range(2):
                        ps = psA.tile([128, 512], F32, tag="psA")
                        for cc in range(NPAIR):
                            nc.tensor.matmul(
                                ps,
                                yT[:, cc, tt * 128 : (tt + 1) * 128],
                                wout_sb[:, cc, nh * 512 : (nh + 1) * 512],
                                start=(cc == 0),
                                stop=(cc == NPAIR - 1),
                            )
                        ot = postage.tile([128, 512], F32, tag="ot")
                        nc.vector.tensor_copy(ot[:], ps[:])
                        nc.sync.dma_start(
                            out_d[tt * 128 : (tt + 1) * 128, nh * 512 : (nh + 1) * 512], ot[:]
                        )

    nc.finalize()
    return nc


def make_host_inputs(x, w_qkv, b_qkv, w_out, b_out, S_=S):
    """Build the 8 per-core input maps (host-side shard/pack/cast)."""
    x = np.asarray(x, dtype=np.float32)
    w_qkv = np.asarray(w_qkv, dtype=np.float32)
    b_qkv = np.asarray(b_qkv, dtype=np.float32)
    w_out = np.asarray(w_out, dtype=np.float32)

    mask = np.where(
        np.arange(896)[None, :] >= np.arange(128)[:, None] + 384, 0.0, NEG
    ).astype(np.float32)

    per_hg = {}
    for hg in range(2):
        wqk = np.empty((D, NPAIR, 2, 128), np.float32)
        bqk = np.empty((128, NPAIR, 2), np.float32)
        for p in range(NPAIR):
            h0, h1 = hg * HPC + 2 * p, hg * HPC + 2 * p + 1
            wqk[:, p, 0, 0:64] = w_qkv[:, h0 * HD : (h0 + 1) * HD]
            wqk[:, p, 0, 64:128] = w_qkv[:, h1 * HD : (h1 + 1) * HD]
            wqk[:, p, 1, 0:64] = w_qkv[:, D + h0 * HD : D + (h0 + 1) * HD]
            wqk[:, p, 1, 64:128] = w_qkv[:, D + h1 * HD : D + (h1 + 1) * HD]
            bqk[0:64, p, 0] = b_qkv[h0 * HD : (h0 + 1) * HD]
            bqk[64:128, p, 0] = b_qkv[h1 * HD : (h1 + 1) * HD]
            bqk[0:64, p, 1] = b_qkv[D + h0 * HD : D + (h0 + 1) * HD]
            bqk[64:128, p, 1] = b_qkv[D + h1 * HD : D + (h1 + 1) * HD]
        wv = w_qkv[:, 2 * D + hg * 512 : 2 * D + (hg + 1) * 512]
        bv = np.broadcast_to(
            b_qkv[2 * D + hg * 512 : 2 * D + (hg + 1) * 512], (128, 512)
        ).copy()
        wout = w_out[hg * 512 : (hg + 1) * 512, :]
        per_hg[hg] = dict(
            wqk=np.ascontiguousarray(wqk.astype(NPBF16)),
            bqk=bqk,
            wv=np.ascontiguousarray(wv.astype(NPBF16)),
            bv=bv,
            wout=np.ascontiguousarray(wout.astype(NPBF16)),
        )

    xT_by_b = [
        np.ascontiguousarray(x[b, :S_].T.astype(NPBF16)) for b in range(B)
    ]
    in_maps = []
    for c in range(8):
        b, hg = c // 2, c % 2
        m = dict(per_hg[hg])
        m["xT"] = xT_by_b[b]
        m["mask"] = mask
        in_maps.append(m)
    return in_maps


_NC_CACHE = {}


def _get_nc(S_=S):
    if S_ not in _NC_CACHE:
        _NC_CACHE[S_] = build_nc(S_)
    return _NC_CACHE[S_]


def kernel(x, w_qkv, b_qkv, w_out, b_out):
    x = np.asarray(x, dtype=np.float32)
    b_out = np.asarray(b_out, dtype=np.float32)
    in_maps = make_host_inputs(x, w_qkv, b_qkv, w_out, b_out)
    nc = _get_nc()
    res = run_bass_kernel_spmd(nc, in_maps, list(range(8))).results
    out = np.empty((B, S, D), np.float32)
    for b in range(B):
        out[b] = res[2 * b]["out"] + res[2 * b + 1]["out"] + b_out[None, :]
    return out
